# revision 1
# baseline (speedup 1.0000x reference)
"""GATNet (2x GATConv + BN + linear + global max pool) on 8 Trainium2 cores.

Self-contained: host-side sharding/scheduling + Bass/Tile kernel + gather.

Strategy (sharding_hint: graph/data parallel over nodes+edges):
  - Layer 1 "pull": hx1 = x @ W1aug computed replicated (full N per core,
    cheap at IN_C=256); edges sharded by dst; per 128-dst block, one-hot
    matmul accumulates exp-weighted messages + softmax denominators in PSUM.
  - Layer 2 "push": edges sharded by src so each core gathers only its LOCAL
    hx2 shard (no 18MB AllGather). Each core accumulates partial dst sums
    for all N nodes, then ONE ReduceScatter (output 1/8 size) lands each
    core's rows. Only a tiny a-coefficient AllGather (240KB) is needed,
    overlapped under the wide mm2.
  - The ReduceScatter is split into two per-core-aligned chunks (A = each
    core's first 640 rows) with dst blocks reordered so RS-A overlaps the
    second half of the edge loop and finalize-A overlaps RS-B.
  - Self-loops are excluded from the push schedule (they'd cost every block
    an extra tile) and applied after the ReduceScatter from local data.
  - One-hot matrices (Mb for aggregation, MbT for the per-block a_d "pick"
    matmul) are precomputed on host and streamed; leaky-relu/exp use Prelu/Exp
    (same ACT table set -> no LoadActFuncSet thrash); the exp broadcast-
    expansion is split ACT/DVE so the big per-edge multiply on DVE is a
    packed (2x-mode) tensor_tensor.
  - BatchNorm stats: per-core partial sums + small AllGather + local 8-way
    reduce (cheaper than AllReduce in latency, biased var as torch BN).
  - Per-graph max pool on device; host combines per-core partials.
"""

import os
import sys
import math
import numpy as np

sys.path.insert(0, "/opt/trn_rl_repo")

# ---------------- problem constants (hardcoded per spec) ----------------
N, E, IN_C, H, C1, OUT_C, B = 10000, 100000, 256, 6, 128, 128, 64
EPS = 1e-5
NEG_SLOPE = 0.2
NCORES = 8
P = 128          # partition dim
GK = 8           # edge tiles per batched dma_gather (>~1024 idxs/call wedges SWDGE)
PG = 8           # graph slots per core (device pooling)
SW = 2           # 128-row subgathers per graph slot (max 256 nodes/graph/core)
F1 = H * C1      # 768
F2 = H * OUT_C   # 768
FA1 = F1 + 2 * H  # written cols of hx1 rows (hx | a_s | a_d)
FA2 = F2 + 2 * H
FU1 = F1 + H      # aggregated cols (msgs | softmax denom)
FU2 = F2 + H

USE_BF16 = os.environ.get("GAT_F32", "0") != "1"
# dma_gather needs elem_size multiple of 256B; rows padded [hx F | a_s | a_d | pad]
FPAD1 = 896 if USE_BF16 else 832     # padded row width of hx tensors
FPAD2 = 896 if USE_BF16 else 832


def wrap_idx16(flat):
    """dma_gather index layout: position i -> [i%16, i//16], replicated to 128 rows."""
    assert len(flat) % 16 == 0
    arr = np.asarray(flat, dtype=np.int16).reshape(-1, 16).T   # [16, n/16]
    return np.tile(arr, (8, 1)).copy()                          # [128, n/16]


# ---------------- host preprocessing ----------------

def _ceil(a, b):
    return (a + b - 1) // b


def build_schedule(counts_per_core_block, block_order=None):
    """Shared (all-core identical) tile schedule from per-(core,block) counts.

    Returns dict with per-block tile counts (max over cores, padded so the
    total is a multiple of GK), tile->block map and start/stop flags.
    block_order: processing order of block ids (default: ascending).
    """
    counts = counts_per_core_block
    nb = counts.shape[1]
    if block_order is None:
        block_order = list(range(nb))
    T_all = np.maximum(1, _ceil(counts.max(axis=0), P))
    T_b = T_all[block_order].copy()
    total = int(T_b.sum())
    T_b[-1] += (-total) % GK          # pad so T is a whole number of gather batches
    total = int(T_b.sum())
    tile_block = np.repeat(np.asarray(block_order), T_b)
    starts = np.zeros(total, dtype=bool)
    stops = np.zeros(total, dtype=bool)
    off = 0
    for k in range(len(block_order)):
        starts[off] = True
        stops[off + T_b[k] - 1] = True
        off += T_b[k]
    return dict(nb=nb, T_b=T_b, T=total, tile_block=tile_block,
                starts=starts, stops=stops, counts=counts,
                block_order=list(block_order))


def pack_core_edges(src_vals, dst_vals, block_lo_fn, sched):
    """Pack this core's (src,dst) edge lists (sorted by dst) into [P, T] arrays.

    block_lo_fn(b) -> dst-range start of block b. Returns (srcT, slotT).
    """
    T, T_b = sched["T"], sched["T_b"]
    src_pad = np.zeros((T, P), dtype=np.int32)
    slot_pad = np.full((T, P), -1, dtype=np.int32)
    off = 0
    for k, b in enumerate(sched["block_order"]):
        lo = block_lo_fn(b)
        hi = lo + P
        e0 = np.searchsorted(dst_vals, lo)
        e1 = np.searchsorted(dst_vals, hi)
        cnt = e1 - e0
        flat_s = src_pad[off:off + T_b[k]].reshape(-1)
        flat_l = slot_pad[off:off + T_b[k]].reshape(-1)
        flat_s[:cnt] = src_vals[e0:e1]
        flat_l[:cnt] = dst_vals[e0:e1] - lo
        off += T_b[k]
    return src_pad.T.copy(), slot_pad.T.copy()   # [P, T]


def build_onehots(slotT, fnp):
    """slotT [P, T] int -> (Mb, MbT) [P, T*P] each in dtype fnp.

    Mb[e, t*P+s]  = (slotT[e,t] == s)   (aggregation lhsT)
    MbT[s, t*P+e] = (slotT[e,t] == s)   (a_d pick lhsT)
    """
    T = slotT.shape[1]
    ar = np.arange(P, dtype=np.int32)
    m3 = (slotT[:, :, None] == ar[None, None, :])        # [e, T, s]
    Mb = m3.astype(fnp).reshape(P, T * P)
    MbT = np.ascontiguousarray(m3.transpose(2, 1, 0)).astype(fnp).reshape(P, T * P)
    return Mb, MbT


def build_pool_layout(ibatch, nper, ncores):
    """Per-core pool gather index lists + slot->graph maps."""
    pool_idx = np.full((ncores, P, PG * SW), nper, dtype=np.int32)  # nper = sentinel row
    slot_graph = np.full((ncores, PG), -1, dtype=np.int64)
    for i in range(ncores):
        ib = ibatch[i * nper:(i + 1) * nper]
        graphs = np.unique(ib)
        if len(graphs) > PG:
            return None, None, False
        for s, g in enumerate(graphs):
            rows = np.nonzero(ib == g)[0].astype(np.int32)
            if len(rows) > SW * P:
                return None, None, False
            slot_graph[i, s] = g
            for sub in range(SW):
                seg = rows[sub * P:(sub + 1) * P]
                pool_idx[i, :len(seg), sub * PG + s] = seg
    return pool_idx, slot_graph, True


def make_aug_weights(W, att_s, att_d, h, c):
    """[K, F] -> [K, F + 2H]: append per-head att projections (a_s | a_d)."""
    K = W.shape[0]
    Wr = W.reshape(K, h, c)
    Was = np.einsum("khc,hc->kh", Wr, att_s)
    Wad = np.einsum("khc,hc->kh", Wr, att_d)
    return np.concatenate([W, Was, Wad], axis=1)


def preprocess(inputs, ncores=NCORES):
    """All host-side index/weight preparation. Returns dict of host arrays."""
    import ml_dtypes
    fnp = ml_dtypes.bfloat16 if USE_BF16 else np.float32
    x = np.asarray(inputs["input_feature"], dtype=np.float32)
    adj = np.asarray(inputs["input_adj"])
    ibatch = np.asarray(inputs["ibatch"]).astype(np.int64)
    n = x.shape[0]
    nper = n // ncores

    loops = np.arange(n, dtype=np.int64)
    src = np.concatenate([adj[0], loops]).astype(np.int64)
    dst = np.concatenate([adj[1], loops]).astype(np.int64)
    is_loop = np.zeros(len(src), dtype=bool)
    is_loop[len(adj[0]):] = True     # the appended self-loops (random dupes stay)
    order = np.argsort(dst, kind="stable")
    src_s = src[order].astype(np.int32)
    dst_s = dst[order].astype(np.int32)
    loop_s = is_loop[order]

    # ----- layer-1 pull schedule: edges by dst; blocks local to each core -----
    nb1 = _ceil(nper, P)
    counts1 = np.zeros((ncores, nb1), dtype=np.int64)
    for i in range(ncores):
        for b in range(nb1):
            lo = i * nper + b * P
            hi = min(i * nper + nper, lo + P)
            counts1[i, b] = np.searchsorted(dst_s, hi) - np.searchsorted(dst_s, lo)
    sched1 = build_schedule(counts1)
    pull = []
    for i in range(ncores):
        e0 = np.searchsorted(dst_s, i * nper)
        e1 = np.searchsorted(dst_s, (i + 1) * nper)
        srcT, slotT = pack_core_edges(
            src_s[e0:e1], dst_s[e0:e1], lambda b, i=i: i * nper + b * P, sched1)
        Mb, MbT = build_onehots(slotT, fnp)
        adrows = i * nper + np.arange(nb1 * P, dtype=np.int32)
        adrows[adrows >= i * nper + nper] = 0      # pad rows: picked by zero MbT
        pull.append(dict(srcT=srcT, slotT=slotT,
                         srcG=wrap_idx16(srcT.T.reshape(-1)),
                         adG=wrap_idx16(adrows), Mb=Mb, MbT=MbT))

    # ----- layer-2 push schedule: edges by src; global dst blocks.
    # Self-loops excluded: their contribution is added from local data after
    # the ReduceScatter (keeps every core's own blocks from costing +1 tile).
    nb2 = _ceil(n, P)
    by_src = [None] * ncores
    counts2 = np.zeros((ncores, nb2), dtype=np.int64)
    for i in range(ncores):
        m = (src_s >= i * nper) & (src_s < (i + 1) * nper) & ~loop_s
        sc, dc = src_s[m] - i * nper, dst_s[m]     # local src idx, global dst
        by_src[i] = (sc, dc)
        counts2[i] = np.bincount(dc // P, minlength=nb2)

    # Split the ReduceScatter into two per-core-aligned chunks so RS-A can
    # overlap the second half of the edge loop (and finalize-A overlaps RS-B).
    # Chunk A = each core's rows [0, RA); chunk B = rows [RA, nper).
    RA = 6 * P                       # rows/core in RS chunk A
    RB = nper - RA

    def classify_segments(b):
        lo, hi = b * P, min(n, (b + 1) * P)
        segs = []                    # (tgt, target_row, offset_in_block, length)
        s = lo
        while s < hi:
            i = s // nper
            aline = i * nper + RA
            if s < aline:
                e = min(hi, aline)
                segs.append(("A", i * RA + (s - i * nper), s - lo, e - s))
            else:
                e = min(hi, (i + 1) * nper)
                segs.append(("B", i * RB + (s - aline), s - lo, e - s))
            s = e
        return segs

    seg_plan = [classify_segments(b) for b in range(nb2)]
    a_blocks = [b for b in range(nb2) if any(t == "A" for t, *_ in seg_plan[b])]
    b_only = [b for b in range(nb2) if b not in set(a_blocks)]
    order2 = a_blocks + b_only
    sched2 = build_schedule(counts2, order2)
    push = []
    for i in range(ncores):
        sc, dc = by_src[i]
        srcT, slotT = pack_core_edges(sc, dc, lambda b: b * P, sched2)
        Mb, MbT = build_onehots(slotT, fnp)
        push.append(dict(srcT=srcT, slotT=slotT,
                         srcG=wrap_idx16(srcT.T.reshape(-1)), Mb=Mb, MbT=MbT))

    pool_idx, slot_graph, pool_ok = build_pool_layout(ibatch, nper, ncores)
    poolG = []
    for i in range(ncores):
        pi = (pool_idx[i] if pool_ok else np.full((P, PG * SW), nper, np.int32))
        poolG.append(wrap_idx16(pi.T.reshape(-1)))       # [128, PG*SW*8]

    W1aug = make_aug_weights(np.asarray(inputs["W1"], np.float32),
                             np.asarray(inputs["att_s1"], np.float32),
                             np.asarray(inputs["att_d1"], np.float32), H, C1)
    W2aug = make_aug_weights(np.asarray(inputs["W2"], np.float32),
                             np.asarray(inputs["att_s2"], np.float32),
                             np.asarray(inputs["att_d2"], np.float32), H, OUT_C)

    def chunked(v, S):  # [F] -> [128, S] (col c = v[c*128:(c+1)*128])
        return np.ascontiguousarray(v.reshape(S, P).T)

    S1, S2 = F1 // P, F2 // P
    host = dict(
        nper=nper, sched1=sched1, sched2=sched2, pull=pull, push=push,
        seg_plan=seg_plan, last_a_block=a_blocks[-1], RA=RA, RB=RB,
        poolG=poolG, pool_idx=pool_idx, slot_graph=slot_graph, pool_ok=pool_ok,
        xT=np.ascontiguousarray(x.T),                    # [IN_C, N]
        W1aug=W1aug, W2aug=W2aug,
        linW=np.asarray(inputs["lin_W"], np.float32),
        b1=np.asarray(inputs["b1"], np.float32),
        b2=np.asarray(inputs["b2"], np.float32),
        linb=np.asarray(inputs["lin_b"], np.float32),
        g1c=chunked(np.asarray(inputs["g1"], np.float32), S1),
        be1c=chunked(np.asarray(inputs["be1"], np.float32), S1),
        g2c=chunked(np.asarray(inputs["g2"], np.float32), S2),
        be2c=chunked(np.asarray(inputs["be2"], np.float32), S2),
        ibatch=ibatch,
    )
    return host


# ---------------- numpy model of the device program ----------------
# Mirrors the per-core device algorithm (same schedule, same op order,
# optional bf16 quantization at the same points) for fast validation.

def _q(a, bf16):
    if not bf16:
        return np.asarray(a, np.float32)
    import ml_dtypes
    return np.asarray(a, dtype=np.float32).astype(ml_dtypes.bfloat16).astype(np.float32)


def _model_edge_tiles(hx_rows, a_d_table, core_edges, sched, F, bf16):
    """Run the shared per-tile edge math. hx_rows: gather source [*, >=F+H]
    (cols F..F+H = a_s). a_d_table: [*, H] indexed by slot block rows.
    Yields (b, psum[P, F+H]) at each block stop."""
    T = sched["T"]
    srcT, slotT = core_edges["srcT"], core_edges["slotT"]
    tile_block, starts, stops = sched["tile_block"], sched["starts"], sched["stops"]
    iota = np.arange(P)
    psum = None
    for t in range(T):
        b = int(tile_block[t])
        if starts[t]:
            psum = np.zeros((P, F + H), dtype=np.float32)
        srci = srcT[:, t].astype(np.int64)
        slot = slotT[:, t]
        g = _q(hx_rows[srci, :F + H], bf16)            # [P, F+H] (hx|a_s)
        adblock = _q(a_d_table[b * P:(b + 1) * P], bf16)  # [<=P, H]
        MbT = (slot[None, :] == iota[:, None]).astype(np.float32)   # [s, e]
        adb_full = np.zeros((P, H), np.float32)
        adb_full[:adblock.shape[0]] = adblock
        ad_e = MbT.T @ adb_full                         # [e, H] pick
        e = g[:, F:F + H] + ad_e
        e = np.maximum(e, NEG_SLOPE * e)
        expc = _q(np.exp(e), bf16)                      # bf16 exp (pre-expand)
        Mb = (slot[:, None] == iota[None, :]).astype(np.float32)    # [e, s]
        rhs = np.concatenate(
            [_q(g[:, :F] * np.repeat(expc, F // H, axis=1), bf16), expc], axis=1)
        psum += Mb.T @ rhs                              # f32 accumulate
        if stops[t]:
            yield b, psum


def _model_finalize_block(psum, bs, F, bias, bf16):
    s = np.maximum(psum[:, F:F + H], 1e-30)
    rs = 1.0 / s
    y = psum[:, :F] * np.repeat(rs, F // H, axis=1)
    y = np.maximum(y + bias[None, :], 0.0)
    return _q(y[:bs], bf16)


def _model_stats(y_sh, S, bf16):
    stats = np.zeros((P, 2 * S), dtype=np.float32)
    ysb = _q(y_sh, bf16)
    for c in range(S):
        blk = ysb[:, c * P:(c + 1) * P]
        stats[:, c] = blk.sum(axis=0)
        stats[:, S + c] = (blk * blk).sum(axis=0)
    return stats


def model_bn_finalize(stats_sum, g_c, be_c, n_total):
    mean = stats_sum[:, :stats_sum.shape[1] // 2] / n_total
    var = stats_sum[:, stats_sum.shape[1] // 2:] / n_total - mean * mean
    rstd = 1.0 / np.sqrt(var + EPS)
    scale = g_c * rstd
    shift = be_c - mean * scale
    return scale, shift


def model_run(inputs, ncores=NCORES, bf16=USE_BF16):
    """Full numpy emulation of the 8-core device program + host combine."""
    host = preprocess(inputs, ncores)
    nper = host["nper"]
    S1, S2 = F1 // P, F2 // P

    xT = _q(host["xT"], bf16)
    W1a = _q(host["W1aug"], bf16)
    W2a = _q(host["W2aug"], bf16)
    linW = _q(host["linW"], bf16)

    # phase A: hx1 replicated
    hx1 = _q(xT.T @ W1a, bf16)                        # [N, F1+2H]

    # layer-1 pull edge phase + stats
    y1 = np.zeros((N, F1), dtype=np.float32)
    stats1 = np.zeros((P, 2 * S1), np.float32)
    for i in range(ncores):
        y_sh = np.zeros((nper, F1), dtype=np.float32)
        for b, psum in _model_edge_tiles(
                hx1, hx1[i * nper:(i + 1) * nper, F1 + H:F1 + 2 * H],
                host["pull"][i], host["sched1"], F1, bf16):
            lo = b * P
            bs = min(nper, lo + P) - lo
            y_sh[lo:lo + bs] = _model_finalize_block(psum, bs, F1, host["b1"], bf16)
        stats1 += _model_stats(y_sh, S1, bf16)
        y1[i * nper:(i + 1) * nper] = _q(y_sh, bf16)
    sc1, sh1 = model_bn_finalize(stats1, host["g1c"], host["be1c"], N)
    scale1 = sc1.T.reshape(-1)   # [F1] feature order (c*128+p)... see chunked()
    # chunked() stores v[c*128+p] at [p, c]; scale applied per feature below
    sc1f = np.ascontiguousarray(sc1.T).reshape(F1)
    sh1f = np.ascontiguousarray(sh1.T).reshape(F1)

    # mm2: hx2 shards (+ compact a2)
    hx2 = np.zeros((N, FA2), dtype=np.float32)
    for i in range(ncores):
        x2 = _q(y1[i * nper:(i + 1) * nper] * sc1f[None, :] + sh1f[None, :], bf16)
        hx2[i * nper:(i + 1) * nper] = _q(x2 @ W2a, bf16)
    a2_full = _q(hx2[:, F2:F2 + 2 * H], bf16)         # [N, 2H] (AllGather'd)

    # layer-2 push edge phase: per-core partials over all N, then reduce
    U2 = np.zeros((N, FU2), dtype=np.float32)
    for i in range(ncores):
        part = np.zeros((N, FU2), dtype=np.float32)
        for b, psum in _model_edge_tiles(
                hx2[i * nper:(i + 1) * nper], a2_full[:, H:2 * H],
                host["push"][i], host["sched2"], F2, bf16):
            lo = b * P
            bs = min(N, lo + P) - lo
            part[lo:lo + bs] = psum[:bs]
        U2 += _q(part, bf16)                           # bf16 partial store + RS sum
    # self-loop contribution, added post-RS from local data
    a_s_all = _q(hx2[:, F2:F2 + H], bf16)
    a_d_all = a2_full[:, H:2 * H]
    es = a_s_all + a_d_all
    es = np.maximum(es, NEG_SLOPE * es)
    qs = np.exp(es)                                    # f32 (SBUF math)
    U2[:, :F2] += _q(_q(hx2[:, :F2], bf16) * np.repeat(qs, OUT_C, axis=1), bf16)
    U2[:, F2:] += qs
    y2 = np.zeros((N, F2), dtype=np.float32)
    stats2 = np.zeros((P, 2 * S2), np.float32)
    for i in range(ncores):
        y_sh = np.zeros((nper, F2), dtype=np.float32)
        for b in range(_ceil(nper, P)):
            lo = b * P
            bs = min(nper, lo + P) - lo
            psum = _q(U2[i * nper + lo:i * nper + lo + bs], bf16)
            pfull = np.zeros((P, FU2), np.float32)
            pfull[:bs] = psum
            y_sh[lo:lo + bs] = _model_finalize_block(pfull, bs, F2, host["b2"], bf16)
        stats2 += _model_stats(y_sh, S2, bf16)
        y2[i * nper:(i + 1) * nper] = _q(y_sh, bf16)
    sc2, sh2 = model_bn_finalize(stats2, host["g2c"], host["be2c"], N)
    sc2f = np.ascontiguousarray(sc2.T).reshape(F2)
    sh2f = np.ascontiguousarray(sh2.T).reshape(F2)

    # final linear
    x3 = np.zeros((N, OUT_C), dtype=np.float32)
    for i in range(ncores):
        xin = _q(y2[i * nper:(i + 1) * nper] * sc2f[None, :] + sh2f[None, :], bf16)
        x3[i * nper:(i + 1) * nper] = xin @ linW + host["linb"][None, :]

    # pooling
    out = np.full((B, OUT_C), -np.inf, dtype=np.float32)
    np.maximum.at(out, host["ibatch"], x3)
    return out


# ================= Bass/Tile device program =================

def build_device_program(host, ncores=NCORES, bf16=USE_BF16, enable_asserts=False,
                         upto=None):
    """Build (and compile) the single SPMD Bass program. Returns nc."""
    import concourse.bass as bass
    import concourse.tile as tile
    from concourse import bacc, mybir
    from concourse.masks import make_identity

    dt = mybir.dt
    fdt = dt.bfloat16 if bf16 else dt.float32
    f32 = dt.float32
    AX = mybir.AxisListType.X
    OP = mybir.AluOpType
    AF = mybir.ActivationFunctionType

    nper = host["nper"]
    sched1, sched2 = host["sched1"], host["sched2"]
    nb1, T1 = sched1["nb"], sched1["T"]
    nb2, T2 = sched2["nb"], sched2["T"]
    S1, S2 = F1 // P, F2 // P
    KC1 = IN_C // P
    groups = [list(range(ncores))]

    class _PhaseStop(Exception):
        pass

    nc = bacc.Bacc("TRN2", target_bir_lowering=False, debug=False,
                   enable_asserts=enable_asserts, num_devices=ncores)

    def inp(name, shape, dtype):
        return nc.dram_tensor(name, shape, dtype, kind="ExternalInput").ap()

    xT_in = inp("xT", [IN_C, N], fdt)
    w1_in = inp("w1aug", [IN_C, FA1], fdt)
    w2_in = inp("w2aug", [F1, FA2], fdt)
    lw_in = inp("linW", [F2, OUT_C], fdt)
    b1_in = inp("b1rep", [P, F1], fdt)
    b2_in = inp("b2rep", [P, F2], fdt)
    lb_in = inp("lbrep", [P, OUT_C], f32)
    g1_in = inp("g1c", [P, S1], f32)
    be1_in = inp("be1c", [P, S1], f32)
    g2_in = inp("g2c", [P, S2], f32)
    be2_in = inp("be2c", [P, S2], f32)
    srcg1_in = inp("srcG1", [P, T1 * 8], dt.int16)
    adg1_in = inp("adG1", [P, (_ceil(nper, P) * P) // 16], dt.int16)
    mb1_in = inp("MbG1", [P, T1 * P], fdt)
    mbt1_in = inp("MbTG1", [P, T1 * P], fdt)
    srcg2_in = inp("srcG2", [P, T2 * 8], dt.int16)
    mb2_in = inp("MbG2", [P, T2 * P], fdt)
    mbt2_in = inp("MbTG2", [P, T2 * P], fdt)
    poolg_in = inp("poolG", [P, PG * SW * 8], dt.int16)

    pooled_out = nc.dram_tensor("pooledT", [P, PG], f32, kind="ExternalOutput").ap()
    x3_out = nc.dram_tensor("x3", [nper, OUT_C], f32, kind="ExternalOutput").ap()

    import contextlib
    with tile.TileContext(nc) as tc:
      with contextlib.suppress(_PhaseStop):
        with tc.tile_pool(name="persist", bufs=1) as pp, \
             tc.tile_pool(name="dram", bufs=1, space="DRAM") as dp:

            # ---- persistent constants in SBUF ----
            ident = pp.tile([P, P], f32, name="ident")
            make_identity(nc, ident[:])
            ident_b = pp.tile([P, P], fdt, name="ident_b")
            make_identity(nc, ident_b[:])
            b1rep = pp.tile([P, F1], fdt, name="b1rep_t")
            nc.sync.dma_start(out=b1rep[:], in_=b1_in[:, :])
            b2rep = pp.tile([P, F2], fdt, name="b2rep_t")
            nc.sync.dma_start(out=b2rep[:], in_=b2_in[:, :])
            lbrep = pp.tile([P, OUT_C], f32, name="lbrep_t")
            nc.sync.dma_start(out=lbrep[:], in_=lb_in[:, :])
            bn_par = {}
            for nm, ap_in in (("g1", g1_in), ("be1", be1_in), ("g2", g2_in), ("be2", be2_in)):
                t = pp.tile([P, ap_in.shape[1]], f32, name=f"{nm}_t")
                nc.sync.dma_start(out=t[:], in_=ap_in[:, :])
                bn_par[nm] = t
            poolg_t = pp.tile([P, PG * SW * 8], dt.int16, name="poolg_t")
            nc.sync.dma_start(out=poolg_t[:], in_=poolg_in[:, :])

            # ---- DRAM scratch ----
            NB2P = nb2 * P                      # N rounded up to whole blocks
            hx1_full = dp.tile([N, FPAD1], fdt, name="hx1_full")
            hx2_shard = dp.tile([nper, FPAD2], fdt, name="hx2_shard")
            a2_loc = dp.tile([nper, 2 * H], fdt, name="a2_loc")
            a2_full = dp.tile([NB2P, 2 * H], fdt, name="a2_full")
            RA, RB = host["RA"], host["RB"]
            u2A = dp.tile([ncores * RA, FU2], fdt, name="u2A")
            u2B = dp.tile([ncores * RB, FU2], fdt, name="u2B")
            u2locA = dp.tile([RA, FU2], fdt, name="u2locA")
            u2locB = dp.tile([RB, FU2], fdt, name="u2locB")
            y1T = dp.tile([F1, nper], fdt, name="y1T")
            y2T = dp.tile([F2, nper], fdt, name="y2T")
            st1_loc = dp.tile([P, 2 * S1], f32, name="st1_loc")
            st1_red = dp.tile([ncores * P, 2 * S1], f32, name="st1_red", addr_space="Shared")
            st2_loc = dp.tile([P, 2 * S2], f32, name="st2_loc")
            st2_red = dp.tile([ncores * P, 2 * S2], f32, name="st2_red", addr_space="Shared")
            x3p = dp.tile([nper + 1, OUT_C], f32, name="x3p")

            def loc_blocksize(b):
                return min(nper, (b + 1) * P) - b * P

            # ====== phase A: hx1 = x @ W1aug (replicated: full N on every core,
            # trades ~8x more PE time for eliminating an 18MB AllGather) ======
            NT_ALL = _ceil(N, P)
            with tc.tile_pool(name="mm1w", bufs=1) as wp, \
                 tc.tile_pool(name="mm1psA", bufs=4, space="PSUM") as qpA, \
                 tc.tile_pool(name="mm1psB", bufs=4, space="PSUM") as qpB, \
                 tc.tile_pool(name="mm1ev", bufs=12) as ep:
                xk = []
                w1k = []
                NQ = 8
                qs_ = [(N * q // NQ, N * (q + 1) // NQ) for q in range(NQ)]
                for kc in range(KC1):
                    wt = wp.tile([P, FA1], fdt, name=f"w1k{kc}", tag=f"w1k{kc}")
                    nc.sync.dma_start(out=wt[:], in_=w1_in[kc * P:(kc + 1) * P, :])
                    w1k.append(wt)
                for kc in range(KC1):
                    xt = wp.tile([P, N], fdt, name=f"xk{kc}", tag=f"xk{kc}")
                    xk.append(xt)
                for q0, q1 in qs_:
                    for kc in range(KC1):
                        nc.sync.dma_start(out=xk[kc][:, q0:q1], in_=xT_in[kc * P:(kc + 1) * P, q0:q1])
                # two 1-bank psum halves, each freed by its own engine's copy,
                # give the PE an 8-deep ring (pstate stays ramped)
                for nt in range(NT_ALL):
                    ns = min(N, (nt + 1) * P) - nt * P
                    psA = qpA.tile([P, 512], f32, name="mm1A", tag="mm1A")
                    psB = qpB.tile([P, FA1 - 512], f32, name="mm1B", tag="mm1B")
                    for kc in range(KC1):
                        nc.tensor.matmul(out=psA[:ns, :],
                                         lhsT=xk[kc][:, nt * P:nt * P + ns],
                                         rhs=w1k[kc][:, 0:512],
                                         start=(kc == 0), stop=(kc == KC1 - 1))
                    for kc in range(KC1):
                        nc.tensor.matmul(out=psB[:ns, :],
                                         lhsT=xk[kc][:, nt * P:nt * P + ns],
                                         rhs=w1k[kc][:, 512:FA1],
                                         start=(kc == 0), stop=(kc == KC1 - 1))
                    ev = ep.tile([P, FA1], fdt, name="mm1ev", tag="mm1ev")
                    nc.scalar.activation(out=ev[:ns, 0:512], in_=psA[:ns, :], func=AF.Copy)
                    nc.vector.tensor_copy(out=ev[:ns, 512:FA1], in_=psB[:ns, :])
                    nc.sync.dma_start(out=hx1_full[nt * P:nt * P + ns, 0:FA1], in_=ev[:ns, :])

            # ================ edge phase (pull l1 / push l2) ================
            def edge_phase(lname, mode, sched, gsrc_ap, FPAD, F,
                           srcg_in_, mb_in_, mbt_in_, ad_setup,
                           brep=None, yT_dram=None, st_sb=None,
                           post_evac=None, sub=None):
                """mode='pull': evacuate softmax-finalized yT + stats.
                   mode='push': evacuate raw psum rows into upart.
                   ad_setup(ip_) -> adview(b): [P, H] SBUF AP of block b's a_d."""
                T = sched["T"]
                tile_block, starts, stops = sched["tile_block"], sched["starts"], sched["stops"]
                S = F // P
                FU = F + H
                if mode == "pull":
                    yT_r = yT_dram.rearrange("(c p) n -> p c n", p=P)
                with tc.tile_pool(name=f"idx_{lname}", bufs=1) as ip_, \
                     tc.tile_pool(name=f"gath_{lname}", bufs=7) as gp, \
                     tc.tile_pool(name=f"mb_{lname}", bufs=6) as mp, \
                     tc.tile_pool(name=f"sm_{lname}", bufs=4) as sp, \
                     tc.tile_pool(name=f"acc_{lname}", bufs=(2 if mode == "pull" else 3),
                                  space="PSUM") as ap_, \
                     tc.tile_pool(name=f"adp_{lname}", bufs=2, space="PSUM") as dp_, \
                     tc.tile_pool(name=f"tp_{lname}", bufs=2, space="PSUM") as tp_, \
                     tc.tile_pool(name=f"ev_{lname}", bufs=4) as ev_:
                    srcg_t = ip_.tile([P, T * 8], dt.int16, name="srcg_t")
                    nc.sync.dma_start(out=srcg_t[:], in_=srcg_in_[:, :])
                    adview = ad_setup(ip_)
                    cur = [None]

                    def evacuate_pull(b, ps):
                        bs = loc_blocksize(b)
                        ssb = sp.tile([P, H], f32, name="ssb", tag="ssb")
                        nc.vector.tensor_scalar_max(out=ssb[:], in0=ps[:, F:F + H], scalar1=1e-30)
                        rs = sp.tile([P, H], f32, name="rs", tag="rs")
                        nc.vector.reciprocal(out=rs[:], in_=ssb[:])
                        y = ev_.tile([P, F], fdt, name="y", tag="y")
                        nc.vector.tensor_tensor(
                            out=y[:, :], in0=ps[:, 0:F],
                            in1=rs[:, :, None].to_broadcast([P, H, F // H]), op=OP.mult)
                        finalize_rows(b, bs, y, yT_r)

                    def finalize_rows(b, bs, y, yT_r_):
                        # y [P, F] fdt = U/s ; add bias, relu, transpose, stats
                        nc.vector.tensor_tensor(out=y[:, :], in0=y[:, :], in1=brep[:, :F], op=OP.add)
                        nc.scalar.activation(out=y[:, :], in_=y[:, :], func=AF.Relu)
                        ytb = ev_.tile([P, S, P], fdt, name="ytb", tag="ytb")
                        for c in range(S):
                            tp = tp_.tile([P, P], fdt, name="tp", tag="tp")
                            nc.tensor.transpose(out=tp[:, :bs], in_=y[:bs, c * P:(c + 1) * P],
                                                identity=ident_b[:bs, :bs])
                            if c % 2 == 0:
                                nc.vector.tensor_copy(out=ytb[:, c, :bs], in_=tp[:, :bs])
                            else:
                                nc.scalar.activation(out=ytb[:, c, :bs], in_=tp[:, :bs],
                                                     func=AF.Copy)
                            # per-(block, chunk) stats land in disjoint columns
                            # of the wide tile -> no serial accumulate chain
                            nc.vector.reduce_sum(out=st_sb[:, c, b:b + 1],
                                                 in_=ytb[:, c, :bs], axis=AX)
                            sq = sp.tile([P, P], f32, name="sq", tag="sq")
                            nc.scalar.activation(out=sq[:, :bs], in_=ytb[:, c, :bs],
                                                 func=AF.Square,
                                                 accum_out=st_sb[:, S + c, b:b + 1])
                        nc.sync.dma_start(out=yT_r_[:, :, b * P:b * P + bs], in_=ytb[:, :, :bs])

                    def evacuate_push(b, ps, alt):
                        bs = min(N, (b + 1) * P) - b * P
                        u8 = ev_.tile([P, FU], fdt, name="u8", tag="u8")
                        nc.scalar.activation(out=u8[:bs, 0:448], in_=ps[:bs, 0:448],
                                             func=AF.Copy)
                        nc.vector.tensor_copy(out=u8[:bs, 448:FU], in_=ps[:bs, 448:FU])
                        for tgt, trow, off, ln in host["seg_plan"][b]:
                            dst_t = u2A if tgt == "A" else u2B
                            nc.sync.dma_start(out=dst_t[trow:trow + ln, :],
                                              in_=u8[off:off + ln, :])
                        if post_evac is not None:
                            post_evac(b)

                    for bi in range(T // GK):
                        t0 = bi * GK
                        gb = gp.tile([P, GK, FPAD], fdt, name="gb", tag="gb")
                        nc.gpsimd.dma_gather(
                            out_ap=gb[:, :, :], in_ap=gsrc_ap[:, :],
                            idxs_ap=srcg_t[:, t0 * 8:(t0 + GK) * 8],
                            num_idxs=GK * P, num_idxs_reg=GK * P, elem_size=FPAD)
                        mb = mp.tile([P, GK, P], fdt, name="mb", tag="mb")
                        nc.sync.dma_start(out=mb[:].rearrange("p a b -> p (a b)"),
                                          in_=mb_in_[:, t0 * P:(t0 + GK) * P])
                        mbt = mp.tile([P, GK, P], fdt, name="mbt", tag="mbt")
                        nc.sync.dma_start(out=mbt[:].rearrange("p a b -> p (a b)"),
                                          in_=mbt_in_[:, t0 * P:(t0 + GK) * P])
                        if sub == "gather":
                            continue
                        # a_d pick: one tiny matmul per subtile into a shared psum
                        adp = dp_.tile([P, GK, H], f32, name="adp", tag="adp")
                        for j in range(GK):
                            t_ = t0 + j
                            b = int(tile_block[t_])
                            nc.tensor.matmul(out=adp[:, j, :], lhsT=mbt[:, j, :],
                                             rhs=adview(b), start=True, stop=True)
                        eb = sp.tile([P, GK, H], f32, name="eb", tag="eb")
                        nc.vector.tensor_tensor(out=eb[:], in0=gb[:, :, F:F + H],
                                                in1=adp[:], op=OP.add)
                        nc.scalar.activation(out=eb[:], in_=eb[:], func=AF.Prelu,
                                             alpha=NEG_SLOPE)
                        expb = sp.tile([P, GK, H], fdt, name="expb", tag="expb")
                        nc.scalar.activation(out=expb[:], in_=eb[:], func=AF.Exp)
                        # broadcast-expand exp weights, split ACT/DVE to balance
                        C = F // H
                        HS = 4
                        expc = sp.tile([P, GK, H, C], fdt, name="expc", tag="expc")
                        nc.scalar.activation(
                            out=expc[:, :, 0:HS, :],
                            in_=expb[:, :, 0:HS, None].to_broadcast([P, GK, HS, C]),
                            func=AF.Copy)
                        nc.vector.tensor_copy(
                            out=expc[:, :, HS:H, :],
                            in_=expb[:, :, HS:H, None].to_broadcast([P, GK, H - HS, C]))
                        if sub == "vec0":
                            continue
                        nc.vector.tensor_tensor(
                            out=gb[:, :, 0:F], in0=gb[:, :, 0:F],
                            in1=expc[:].rearrange("p a b c -> p a (b c)"),
                            op=OP.mult)
                        nc.vector.tensor_copy(out=gb[:, :, F:F + H], in_=expb[:])
                        if sub == "vec":
                            continue
                        for j in range(GK):
                            t_ = t0 + j
                            b = int(tile_block[t_])
                            if starts[t_]:
                                cur[0] = ap_.tile([P, FU], f32, name="acc", tag="acc")
                            ps = cur[0]
                            for c0, c1 in ((0, 512), (512, FU)):
                                nc.tensor.matmul(out=ps[:, c0:c1], lhsT=mb[:, j, :],
                                                 rhs=gb[:, j, c0:c1],
                                                 start=bool(starts[t_]), stop=bool(stops[t_]))
                            if stops[t_]:
                                if sub == "mm":
                                    cur[0] = None
                                elif mode == "pull":
                                    evacuate_pull(b, ps)
                                else:
                                    evacuate_push(b, ps, b)

            # ---- BN stats reduce + finalize -> scale/shift tiles ----
            def bn_reduce(lname, S, st_sb, st_loc, st_red, g_t, be_t):
                stflat = pp.tile([P, 2 * S], f32, name=f"stflat_{lname}")
                nc.vector.reduce_sum(out=stflat[:], in_=st_sb[:], axis=AX)
                nc.sync.dma_start(out=st_loc[:, :], in_=stflat[:])
                # AllGather + local 8-way sum: ~16us vs AllReduce's ~28us
                # (the cost model charges AllReduce 1.875x on the same bytes)
                nc.gpsimd.collective_compute(
                    "AllGather", OP.bypass, replica_groups=groups,
                    ins=[st_loc[:, :].opt()], outs=[st_red[:, :].opt()])
                sredg = pp.tile([P, 2 * S, ncores], f32, name=f"sredg_{lname}")
                nc.sync.dma_start(
                    out=sredg[:], in_=st_red.rearrange("(g p) s -> p s g", p=P))
                sred = pp.tile([P, 2 * S], f32, name=f"sred_{lname}")
                nc.vector.reduce_sum(out=sred[:], in_=sredg[:], axis=AX)
                mean = pp.tile([P, S], f32, name=f"mean_{lname}")
                nc.scalar.activation(out=mean[:], in_=sred[:, 0:S], func=AF.Copy, scale=1.0 / N)
                msq = pp.tile([P, S], f32, name=f"msq_{lname}")
                nc.scalar.activation(out=msq[:], in_=mean[:], func=AF.Square)
                var = pp.tile([P, S], f32, name=f"var_{lname}")
                nc.scalar.activation(out=var[:], in_=sred[:, S:2 * S], func=AF.Copy, scale=1.0 / N)
                nc.vector.tensor_sub(out=var[:], in0=var[:], in1=msq[:])
                nc.vector.tensor_scalar_add(out=var[:], in0=var[:], scalar1=EPS)
                sd = pp.tile([P, S], f32, name=f"sd_{lname}")
                nc.scalar.activation(out=sd[:], in_=var[:], func=AF.Sqrt)
                rstd = pp.tile([P, S], f32, name=f"rstd_{lname}")
                nc.vector.reciprocal(out=rstd[:], in_=sd[:])
                scale_t = pp.tile([P, S], f32, name=f"scale_{lname}")
                nc.vector.tensor_mul(out=scale_t[:], in0=g_t[:], in1=rstd[:])
                tmp = pp.tile([P, S], f32, name=f"tmp_{lname}")
                nc.vector.tensor_mul(out=tmp[:], in0=mean[:], in1=scale_t[:])
                shift_t = pp.tile([P, S], f32, name=f"shift_{lname}")
                nc.vector.tensor_sub(out=shift_t[:], in0=be_t[:], in1=tmp[:])
                return scale_t, shift_t

            if upto == "mm1":
                raise _PhaseStop()

            # ================ layer-1 edge phase (pull) ================
            stats1 = pp.tile([P, 2 * S1, nb1], f32, name="stats_l1")
            sub1 = upto[3:] if (upto or "").startswith("l1:") else None

            def ad_setup_l1(ip_):
                # one-shot gather of this core's block rows (indices are
                # per-core input data), a-window cols [F1, F1+128)
                a1blk = ip_.tile([P, nb1, P], fdt, name="a1blk")
                adg_t = ip_.tile([P, (nb1 * P) // 16], dt.int16, name="adg_t")
                nc.sync.dma_start(out=adg_t[:], in_=adg1_in[:, :])
                hhn = nb1 // 2
                for hh in range(2):
                    nc.gpsimd.dma_gather(
                        out_ap=a1blk[:, hh * hhn:(hh + 1) * hhn, :],
                        in_ap=hx1_full[:, F1:F1 + P],
                        idxs_ap=adg_t[:, hh * hhn * 8:(hh + 1) * hhn * 8],
                        num_idxs=hhn * P, num_idxs_reg=hhn * P, elem_size=P,
                        elem_step=FPAD1)
                return lambda b: a1blk[:, b, H:2 * H]

            edge_phase("l1", "pull", sched1, hx1_full, FPAD1, F1,
                       srcg1_in, mb1_in, mbt1_in, ad_setup_l1,
                       brep=b1rep, yT_dram=y1T, st_sb=stats1, sub=sub1)
            if sub1 is not None:
                raise _PhaseStop()
            sc1, sh1 = bn_reduce("l1", S1, stats1, st1_loc, st1_red,
                                 bn_par["g1"], bn_par["be1"])

            if upto == "l1":
                raise _PhaseStop()

            # ================ mm2: narrow a-cols pass + AllGather, wide pass ===
            y1T_r = y1T.rearrange("(c p) n -> p c n", p=P)
            with tc.tile_pool(name="mm2w", bufs=1) as wp, \
                 tc.tile_pool(name="mm2lhs", bufs=1) as lp, \
                 tc.tile_pool(name="mm2ps", bufs=2, space="PSUM") as qp, \
                 tc.tile_pool(name="mm2aps", bufs=2, space="PSUM") as aqp, \
                 tc.tile_pool(name="mm2ev", bufs=5) as ep:
                w2k = []
                for kc in range(S1):
                    wt = wp.tile([P, FA2], fdt, name=f"w2k{kc}", tag=f"w2k{kc}")
                    nc.sync.dma_start(out=wt[:], in_=w2_in[kc * P:(kc + 1) * P, :])
                    w2k.append(wt)
                lall = []
                for nt in range(nb1):
                    ns = loc_blocksize(nt)
                    lt_all = lp.tile([P, S1, P], fdt, name=f"lall2_{nt}", tag=f"lall2_{nt}")
                    nc.sync.dma_start(out=lt_all[:, :, :ns], in_=y1T_r[:, :, nt * P:nt * P + ns])
                    aps = aqp.tile([P, 2 * H], f32, name="a2acc", tag="a2acc")
                    for kc in range(S1):
                        lt = lt_all[:, kc, :ns]
                        nc.vector.scalar_tensor_tensor(
                            out=lt, in0=lt, scalar=sc1[:, kc:kc + 1],
                            in1=sh1[:, kc:kc + 1].to_broadcast([P, ns]),
                            op0=OP.mult, op1=OP.add)
                        nc.tensor.matmul(out=aps[:ns, :], lhsT=lt,
                                         rhs=w2k[kc][:, F2:F2 + 2 * H],
                                         start=(kc == 0), stop=(kc == S1 - 1))
                    aev = ep.tile([P, 2 * H], fdt, name="a2ev", tag="a2ev")
                    nc.vector.tensor_copy(out=aev[:ns, :], in_=aps[:ns, :])
                    nc.sync.dma_start(out=a2_loc[nt * P:nt * P + ns, :], in_=aev[:ns, :])
                    lall.append(lt_all)
                nc.gpsimd.collective_compute(
                    "AllGather", OP.bypass, replica_groups=groups,
                    ins=[a2_loc[:, :].opt()], outs=[a2_full[0:N, :].opt()])
                if NB2P > N:   # zero the pad rows so the a_d pick never sees NaN
                    zpad = wp.tile([NB2P - N, 2 * H], fdt, name="zpad")
                    nc.gpsimd.memset(zpad[:], 0.0)
                    nc.sync.dma_start(out=a2_full[N:NB2P, :], in_=zpad[:])
                for nt in range(nb1):
                    ns = loc_blocksize(nt)
                    ps = qp.tile([P, F2 + H], f32, name="mm2acc", tag="mm2acc")
                    for kc in range(S1):
                        for c0, c1 in ((0, 512), (512, F2 + H)):
                            nc.tensor.matmul(out=ps[:ns, c0:c1], lhsT=lall[nt][:, kc, :ns],
                                             rhs=w2k[kc][:, c0:c1],
                                             start=(kc == 0), stop=(kc == S1 - 1))
                    ev = ep.tile([P, F2 + H], fdt, name="mm2ev", tag="mm2ev")
                    nc.scalar.activation(out=ev[:ns, :], in_=ps[:ns, :], func=AF.Copy)
                    nc.sync.dma_start(out=hx2_shard[nt * P:nt * P + ns, 0:F2 + H], in_=ev[:ns, :])

            if upto == "mm2":
                raise _PhaseStop()

            # ================ layer-2 edge phase (push) + ReduceScatter ========
            sub2 = upto[3:] if (upto or "").startswith("l2:") else None

            def ad_setup_l2(ip_):
                a2blk = ip_.tile([P, nb2, 2 * H], fdt, name="a2blk")
                nc.sync.dma_start(
                    out=a2blk[:], in_=a2_full.rearrange("(b p) h -> p b h", p=P))
                return lambda b: a2blk[:, b, H:2 * H]

            def post_evac_l2(b):
                if b == host["last_a_block"]:
                    # RS chunk A fires mid-loop, overlapping the B half
                    nc.gpsimd.collective_compute(
                        "ReduceScatter", OP.add, replica_groups=groups,
                        ins=[u2A[:, :].opt()], outs=[u2locA[:, :].opt()])

            edge_phase("l2", "push", sched2, hx2_shard, FPAD2, F2,
                       srcg2_in, mb2_in, mbt2_in, ad_setup_l2,
                       post_evac=post_evac_l2, sub=sub2)
            if sub2 is not None:
                raise _PhaseStop()
            nc.gpsimd.collective_compute(
                "ReduceScatter", OP.add, replica_groups=groups,
                ins=[u2B[:, :].opt()], outs=[u2locB[:, :].opt()])

            if upto == "rs2":
                raise _PhaseStop()

            # ---- layer-2 finalize: softmax div + bias + relu + stats ----
            stats2 = pp.tile([P, 2 * S2, nb1], f32, name="stats_l2")
            with tc.tile_pool(name="fin2", bufs=3) as ev_, \
                 tc.tile_pool(name="fin2s", bufs=3) as sp, \
                 tc.tile_pool(name="fin2tp", bufs=2, space="PSUM") as tp_:
                y2T_r = y2T.rearrange("(c p) n -> p c n", p=P)
                for b in range(nb1):
                    bs = loc_blocksize(b)
                    u = ev_.tile([P, FU2], fdt, name="u2b", tag="u2b")
                    if b < RA // P:
                        u_src = u2locA[b * P:b * P + bs, :]
                    else:
                        u_src = u2locB[b * P - RA:b * P - RA + bs, :]
                    nc.sync.dma_start(out=u[:bs, :], in_=u_src)
                    # self-loop contribution from purely local data
                    hxr = ev_.tile([P, F2 + H], fdt, name="hxr", tag="hxr")
                    nc.sync.dma_start(out=hxr[:bs, :], in_=hx2_shard[b * P:b * P + bs, 0:F2 + H])
                    a2b = sp.tile([P, H], fdt, name="a2b", tag="a2b")
                    nc.sync.dma_start(out=a2b[:bs, :], in_=a2_loc[b * P:b * P + bs, H:2 * H])
                    ebs = sp.tile([P, H], f32, name="ebs", tag="ebs")
                    nc.vector.tensor_tensor(out=ebs[:bs, :], in0=hxr[:bs, F2:F2 + H],
                                            in1=a2b[:bs, :], op=OP.add)
                    nc.scalar.activation(out=ebs[:bs, :], in_=ebs[:bs, :], func=AF.Prelu,
                                         alpha=NEG_SLOPE)
                    qs = sp.tile([P, H], f32, name="qs", tag="qs")
                    nc.scalar.activation(out=qs[:bs, :], in_=ebs[:bs, :], func=AF.Exp)
                    m1 = ev_.tile([P, F2], fdt, name="m1", tag="m1")
                    nc.vector.tensor_tensor(
                        out=m1[:bs, :], in0=hxr[:bs, 0:F2],
                        in1=qs[:bs, :, None].to_broadcast([bs, H, OUT_C]), op=OP.mult)
                    usum = ev_.tile([P, F2], fdt, name="usum", tag="usum")
                    nc.vector.tensor_tensor(out=usum[:bs, :], in0=u[:bs, 0:F2],
                                            in1=m1[:bs, :], op=OP.add)
                    ssb = sp.tile([P, H], f32, name="ssb2", tag="ssb2")
                    nc.vector.tensor_tensor(out=ssb[:bs, :], in0=u[:bs, F2:FU2],
                                            in1=qs[:bs, :], op=OP.add)
                    nc.vector.tensor_scalar_max(out=ssb[:bs, :], in0=ssb[:bs, :],
                                                scalar1=1e-30)
                    rs = sp.tile([P, H], f32, name="rs2", tag="rs2")
                    nc.vector.reciprocal(out=rs[:bs, :], in_=ssb[:bs, :])
                    y = ev_.tile([P, F2], fdt, name="y2b", tag="y2b")
                    nc.vector.tensor_tensor(
                        out=y[:bs, :], in0=usum[:bs, :],
                        in1=rs[:bs, :, None].to_broadcast([bs, H, OUT_C]), op=OP.mult)
                    nc.vector.tensor_tensor(out=y[:bs, :], in0=y[:bs, :],
                                            in1=b2rep[:bs, :], op=OP.add)
                    nc.scalar.activation(out=y[:bs, :], in_=y[:bs, :], func=AF.Relu)
                    ytb = ev_.tile([P, S2, P], fdt, name="ytb2", tag="ytb2")
                    for c in range(S2):
                        tp = tp_.tile([P, P], fdt, name="tp2", tag="tp2")
                        nc.tensor.transpose(out=tp[:, :bs], in_=y[:bs, c * P:(c + 1) * P],
                                            identity=ident_b[:bs, :bs])
                        if c % 2 == 0:
                            nc.vector.tensor_copy(out=ytb[:, c, :bs], in_=tp[:, :bs])
                        else:
                            nc.scalar.activation(out=ytb[:, c, :bs], in_=tp[:, :bs],
                                                 func=AF.Copy)
                        nc.vector.reduce_sum(out=stats2[:, c, b:b + 1],
                                             in_=ytb[:, c, :bs], axis=AX)
                        sq = sp.tile([P, P], f32, name="sq2", tag="sq2")
                        nc.scalar.activation(out=sq[:, :bs], in_=ytb[:, c, :bs],
                                             func=AF.Square,
                                             accum_out=stats2[:, S2 + c, b:b + 1])
                    nc.sync.dma_start(out=y2T_r[:, :, b * P:b * P + bs], in_=ytb[:, :, :bs])
            sc2, sh2 = bn_reduce("l2", S2, stats2, st2_loc, st2_red,
                                 bn_par["g2"], bn_par["be2"])

            if upto == "l2":
                raise _PhaseStop()

            # ================ phase E: x3 = bn(y2) @ linW + lb ================
            y2T_r2 = y2T.rearrange("(c p) n -> p c n", p=P)
            with tc.tile_pool(name="mm3w", bufs=1) as wp, \
                 tc.tile_pool(name="mm3lhs", bufs=3) as lp, \
                 tc.tile_pool(name="mm3ps", bufs=2, space="PSUM") as qp, \
                 tc.tile_pool(name="mm3ev", bufs=3) as ep:
                lwk = []
                for kc in range(S2):
                    wt = wp.tile([P, OUT_C], fdt, name=f"lwk{kc}", tag=f"lwk{kc}")
                    nc.sync.dma_start(out=wt[:], in_=lw_in[kc * P:(kc + 1) * P, :])
                    lwk.append(wt)
                sent = wp.tile([1, OUT_C], f32, name="sent")
                nc.gpsimd.memset(sent[:], -1e30)
                nc.sync.dma_start(out=x3p[nper:nper + 1, :], in_=sent[:])
                for nt in range(nb1):
                    ns = loc_blocksize(nt)
                    lall3 = lp.tile([P, S2, P], fdt, name="lall3", tag="lall3")
                    nc.sync.dma_start(out=lall3[:, :, :ns], in_=y2T_r2[:, :, nt * P:nt * P + ns])
                    ps = qp.tile([P, OUT_C], f32, name="mm3acc", tag="mm3acc")
                    for kc in range(S2):
                        lt = lall3[:, kc, :ns]
                        nc.vector.scalar_tensor_tensor(
                            out=lt, in0=lt, scalar=sc2[:, kc:kc + 1],
                            in1=sh2[:, kc:kc + 1].to_broadcast([P, ns]),
                            op0=OP.mult, op1=OP.add)
                        nc.tensor.matmul(out=ps[:ns, :], lhsT=lt, rhs=lwk[kc][:, :],
                                         start=(kc == 0), stop=(kc == S2 - 1))
                    x3sb = ep.tile([P, OUT_C], f32, name="x3sb", tag="x3sb")
                    nc.vector.tensor_tensor(out=x3sb[:ns, :], in0=ps[:ns, :],
                                            in1=lbrep[:ns, :], op=OP.add)
                    nc.sync.dma_start(out=x3p[nt * P:nt * P + ns, :], in_=x3sb[:ns, :])
                nc.sync.dma_start(out=x3_out[:, :], in_=x3p[0:nper, :])

            # ================ phase F: per-graph max pool ================
            if upto == "mm3":
                raise _PhaseStop()
            with tc.tile_pool(name="pool", bufs=1) as gp, \
                 tc.tile_pool(name="poolps", bufs=2, space="PSUM") as tp_:
                pg = gp.tile([P, PG * SW, OUT_C], f32, name="pg")
                half = PG * SW // 2
                for hh in range(2):
                    nc.gpsimd.dma_gather(
                        out_ap=pg[:, hh * half:(hh + 1) * half, :], in_ap=x3p[:, :],
                        idxs_ap=poolg_t[:, hh * half * 8:(hh + 1) * half * 8],
                        num_idxs=half * P, num_idxs_reg=half * P, elem_size=OUT_C)
                pcols = gp.tile([P, PG * SW], f32, name="pcols")
                for j in range(PG * SW):
                    tp = tp_.tile([P, P], f32, name="ptp", tag="ptp")
                    nc.tensor.transpose(out=tp[:OUT_C, :], in_=pg[:, j, :], identity=ident[:])
                    nc.vector.reduce_max(out=pcols[:, j:j + 1], in_=tp[:, :], axis=AX)
                pooled_sb = gp.tile([P, PG], f32, name="pooled_sb")
                nc.vector.tensor_max(out=pooled_sb[:], in0=pcols[:, 0:PG],
                                     in1=pcols[:, PG:2 * PG])
                nc.sync.dma_start(out=pooled_out[:, :], in_=pooled_sb[:])

    nc.compile()
    return nc


def make_in_maps(host, ncores=NCORES, bf16=USE_BF16):
    import ml_dtypes
    fnp = ml_dtypes.bfloat16 if bf16 else np.float32
    nper = host["nper"]
    shared = dict(
        w1aug=host["W1aug"].astype(fnp),
        w2aug=host["W2aug"].astype(fnp),
        linW=host["linW"].astype(fnp),
        b1rep=np.tile(host["b1"], (P, 1)).astype(fnp),
        b2rep=np.tile(host["b2"], (P, 1)).astype(fnp),
        lbrep=np.tile(host["linb"], (P, 1)).astype(np.float32),
        g1c=host["g1c"], be1c=host["be1c"], g2c=host["g2c"], be2c=host["be2c"],
    )
    in_maps = []
    xT_b = host["xT"].astype(fnp)         # replicated full xT
    for i in range(ncores):
        m = dict(shared)
        m["xT"] = xT_b
        m["srcG1"] = host["pull"][i]["srcG"]
        m["adG1"] = host["pull"][i]["adG"]
        m["MbG1"] = host["pull"][i]["Mb"]
        m["MbTG1"] = host["pull"][i]["MbT"]

        m["srcG2"] = host["push"][i]["srcG"]
        m["MbG2"] = host["push"][i]["Mb"]
        m["MbTG2"] = host["push"][i]["MbT"]

        m["poolG"] = host["poolG"][i]
        in_maps.append(m)
    return in_maps


def postprocess(results, host, ncores=NCORES):
    nper = host["nper"]
    out = np.full((B, OUT_C), -np.inf, dtype=np.float32)
    if host["pool_ok"]:
        for i in range(ncores):
            pt = results[i]["pooledT"]          # [128, PG]
            for s in range(PG):
                g = host["slot_graph"][i, s]
                if g >= 0:
                    out[g] = np.maximum(out[g], pt[:OUT_C, s])
    else:
        x3 = np.concatenate([results[i]["x3"] for i in range(ncores)], axis=0)
        np.maximum.at(out, host["ibatch"], x3)
    return out


def kernel(**inputs):
    from concourse.bass_utils import run_bass_kernel_spmd
    host = preprocess(inputs, NCORES)
    nc = build_device_program(host, NCORES, USE_BF16)
    in_maps = make_in_maps(host, NCORES, USE_BF16)
    res = run_bass_kernel_spmd(nc, in_maps, core_ids=list(range(NCORES)))
    return postprocess(res.results, host, NCORES)

